# revision 23
# baseline (speedup 1.0000x reference)
"""Trainium2 Bass kernel for the binarized ConvNet (nn_ConvNet_81501299409071).

Data-parallel over batch: 8192 images -> 8 NeuronCores x 1024 images.

Device pipeline (feature-major: features on partitions, batch on free dim),
everything is a matmul against exactly-representable +-1 Toeplitz weight
matrices; the DoReFa binarization scale E is folded into the post-matmul
activation ops (relu(acc*E + b)).

The conv/fc matmul path runs in bf16 (the +-1 weights are exact in bf16;
images round at ~0.4% rel which is far inside the 2e-2 gate). This halves
HBM traffic and doubles DVE throughput on the SBUF-side pooling ops. The
softmax tail stays fp32/f32r.

  conv1 5x5 (1->10ch):  6x2 input tiles [128=(8 rows x 16 cols), N=512],
      4 matmuls of M=120 per tile; M packed as (row-in-pair, ch, col-pair)
      so 2x2 maxpool is two full-width tensor_max ops whose outputs land
      directly in conv2-ready [120=(half, ch, col), N] tiles.
  conv2 3x3 (10->20ch): per output row, 3 accumulating K=120 matmuls x 2
      output-channel halves.
  fc1 2000->50: 20 accumulating K=100 matmuls (one per conv2 relu tile).
  fc2 50->10 + log_softmax: exp/ln on ACT, partition sum / broadcast via
      tiny ones-matmuls, final subtract on DVE.

Startup: the PE is kept busy with tiny self-dependent warm-up matmuls while
the first input/weight DMAs land, so the HAM clock gate is already at 8/8
when real matmuls start. Input is pre-tiled on the host so every rhs DMA is
a single contiguous block, issued alternately on both HWDGE queues.
"""
import os
import numpy as np
import ml_dtypes

import concourse.bass as bass
import concourse.tile as tile
from concourse import bacc, mybir
from concourse.bass_utils import run_bass_kernel_spmd

F32 = mybir.dt.float32
F32R = mybir.dt.float32r
BF16 = mybir.dt.bfloat16
NPBF16 = ml_dtypes.bfloat16

N_CORES = 8
B_TOTAL = 8192
BC = B_TOTAL // N_CORES  # 1024 images per core
N = 512                  # batch tile (free dim / PSUM bank)
N_TILES = BC // N

LAST_EXEC_TIME_NS = None
LAST_RESULTS = None

# ---------------------------------------------------------------------------
# weight blobs: wb16 (bf16) holds every matmul lhsT for conv1/conv2/fc1;
# wb32 (f32r) holds the fp32 tail operands + bias columns.
# ---------------------------------------------------------------------------
_off16 = 0
def _t16(n):
    global _off16
    c = _off16
    _off16 += n
    return c

_off32 = 0
def _t32(n):
    global _off32
    c = _off32
    _off32 += n
    return c

C_LHST1 = [[_t16(128) for _par in range(2)] for _jp in range(2)]    # [j2][par]
C_LHST2 = [[_t16(100) for _s in range(2)] for _dy in range(3)]      # [dy][s]
C_LHSTF1 = [[_t16(50) for _s in range(2)] for _yo in range(10)]     # [yo][s]
W16_COLS = _off16

C_LHSTF2 = _t32(10)           # K=50
C_ONES_ROW = _t32(10)         # [1,10] ones (broadcast lhsT)
C_ONES_COL = _t32(1)          # [10,1] ones (partition-sum lhsT)
C_B1 = _t32(1)                # [120,1]
C_B2 = [_t32(1) for _s in range(2)]  # [100,1] each
C_BF1 = _t32(1)               # [50,1]
C_BF2 = _t32(1)               # [10,1]
W32_COLS = _off32


def _host_prep(inputs):
    """Binarize weights, build +-1 Toeplitz matrices + bias columns packed
    into the two weight blobs, and the E scales."""
    w1, b1 = inputs["w1"], inputs["b1"]
    w2, b2 = inputs["w2"], inputs["b2"]
    fw1, fb1 = inputs["fw1"], inputs["fb1"]
    fw2, fb2 = inputs["fw2"], inputs["fb2"]

    scales = {
        "E1": float(np.mean(np.abs(w1))),
        "E2": float(np.mean(np.abs(w2))),
        "Ef1": float(np.mean(np.abs(fw1))),
        "Ef2": float(np.mean(np.abs(fw2))),
    }
    s1 = np.sign(w1).astype(np.float32)
    s2 = np.sign(w2).astype(np.float32)
    sf1 = np.sign(fw1).astype(np.float32)
    sf2 = np.sign(fw2).astype(np.float32)

    wb16 = np.zeros((128, W16_COLS), np.float32)
    wb32 = np.zeros((128, W32_COLS), np.float32)

    # conv1 Toeplitz [j2][par] (j2 = row-in-pair): M m = jp*64 + c*10 + oc
    # ((c, oc) col-major within each jp half so the pooled halves are
    # partition-contiguous and the conv2 row repack is a plain
    # partition-shifted DMA copy; pads zero)
    for j2 in range(2):
        for par in range(2):
            blk = np.zeros((128, 128), np.float32)
            for jp in range(2):
                j = 2 * jp + j2
                for oc in range(10):
                    for c in range(6):
                        m = jp * 64 + c * 10 + oc
                        xo = 2 * c + par
                        for dy in range(5):
                            r = j + dy
                            for dx in range(5):
                                xi = xo + dx
                                blk[r * 16 + xi, m] = s1[oc, 0, dy, dx]
            co = C_LHST1[j2][par]
            wb16[:, co:co + 128] = blk

    # conv2 Toeplitz [dy][s]: rhs is a full-width pooled row
    # K k = xi*10 + ci (12 cols x 10 ch); M m = oci*10 + xo
    for dy in range(3):
        for s_ in range(2):
            blk = np.zeros((120, 100), np.float32)
            for xi in range(12):
                for ci in range(10):
                    for oci in range(10):
                        for xo in range(10):
                            dx = xi - xo
                            if 0 <= dx < 3:
                                blk[xi * 10 + ci, oci * 10 + xo] = \
                                    s2[10 * s_ + oci, ci, dy, dx]
            co = C_LHST2[dy][s_]
            wb16[0:120, co:co + 100] = blk

    # fc1 [yo][s]: K p = oci*10+xo -> f = (10s+oci)*100 + yo*10 + xo
    for yo in range(10):
        for s in range(2):
            blk = np.zeros((100, 50), np.float32)
            for oci in range(10):
                for xo in range(10):
                    f = (10 * s + oci) * 100 + yo * 10 + xo
                    blk[oci * 10 + xo, :] = sf1[:, f]
            co = C_LHSTF1[yo][s]
            wb16[0:100, co:co + 50] = blk

    wb32[0:50, C_LHSTF2:C_LHSTF2 + 10] = sf2.T
    wb32[0, C_ONES_ROW:C_ONES_ROW + 10] = 1.0
    wb32[0:10, C_ONES_COL] = 1.0

    # bias columns, pre-divided by the accumulated binarization scales so
    # every bias+relu runs unscaled (relu(acc + b')) on any engine; the one
    # true scale Etot is applied at the logits.
    E1, E2, Ef1 = scales["E1"], scales["E2"], scales["Ef1"]
    b1v = np.zeros(128, np.float32)
    for jp in range(2):
        for c in range(6):
            for oc in range(10):
                b1v[jp * 64 + c * 10 + oc] = b1[oc] / E1
    wb32[:, C_B1] = b1v
    for s in range(2):
        b2v = np.repeat(b2[10 * s:10 * s + 10], 10).astype(np.float32)
        wb32[0:100, C_B2[s]] = b2v / (E1 * E2)
    wb32[0:50, C_BF1] = fb1 / (E1 * E2 * Ef1)
    wb32[0:10, C_BF2] = fb2
    return wb16.astype(NPBF16), wb32, scales


# tuning knobs (engine splits / pool sizing), overridable for sweeps
CFG = {
    "rowmax_gp_mod": 2,     # rowmax i -> gpsimd when i % mod < thr
    "rowmax_gp_thr": 0,     # (gpsimd TENSOR_TENSOR rejected by trn2 ISA)
    "cme_dve_mod": 6,       # u-extract i -> DVE when i % mod == mod-1, else ACT
    "a2_dve_mod": 4,        # a2 relu -> DVE when (2*yo+s) % mod == mod-1
    "p1_bufs": 2,
    "rhs_bufs": 16,
    "n_warm": 24,           # PE warm-up matmuls issued during startup DMA wait
}


def build_program(scales, n_tiles=N_TILES, bc=BC, cfg=None, repeat=1):
    """Build the single-core SPMD bass program."""
    cfg = {**CFG, **(cfg or {})}
    Etot = scales["E1"] * scales["E2"] * scales["Ef1"] * scales["Ef2"]
    Relu = mybir.ActivationFunctionType.Relu
    Ident = mybir.ActivationFunctionType.Identity
    Exp = mybir.ActivationFunctionType.Exp
    Ln = mybir.ActivationFunctionType.Ln
    Add = mybir.AluOpType.add
    Max = mybir.AluOpType.max

    nc = bacc.Bacc("TRN2", target_bir_lowering=False, debug=False)
    # pre-tiled input: one contiguous [128, N] block per (nt, t, h)
    xt = nc.dram_tensor("xt", [n_tiles, 128, 12 * N], BF16,
                        kind="ExternalInput").ap()
    wb16d = nc.dram_tensor("wb16", [128, W16_COLS], BF16,
                           kind="ExternalInput").ap()
    wb32d = nc.dram_tensor("wb32", [128, W32_COLS], F32R,
                           kind="ExternalInput").ap()
    out = nc.dram_tensor("out", [10, bc], F32, kind="ExternalOutput").ap()

    with tile.TileContext(nc) as tc:
        with tc.tile_pool(name="wpool", bufs=1) as wpool, \
             tc.tile_pool(name="sb", bufs=1) as sb, \
             tc.tile_pool(name="ps", bufs=1, space="PSUM") as ps:

            # --- PE warm-up: tiny self-contained matmuls with no DMA deps
            # keep the PE busy from the first instruction, so the HAM clock
            # gate is released (~3.4us of activity) before real work lands.
            # N=128 each so ~40 of them bridge the ~5us startup DMA window.
            warm = sb.tile([128, 128], BF16, tag="warm")
            nc.vector.memset(warm[:], 1.0)
            wps = ps.tile([8, 128], F32, tag="p2", bufs=2)
            for _k in range(cfg["n_warm"]):
                nc.tensor.matmul(wps[:], warm[:, 0:8], warm[:],
                                 start=True, stop=True)

            c1w = 4 * 128
            wbc1 = wpool.tile([128, c1w], BF16, tag="wbc1")
            wbr = wpool.tile([128, W16_COLS - c1w], BF16, tag="wbr")
            wb32 = wpool.tile([128, W32_COLS], F32R, tag="wb32")
            # conv1 weights in their own tile so its LDWs only wait on this
            # small DMA; the rest is issued AFTER tile 0's rhs DMAs (below)
            # so it doesn't delay them on the scalar HWDGE queue.
            nc.scalar.dma_start(wbc1[:], wb16d[:, 0:c1w])
            nc.scalar.dma_start(wb32[:], wb32d[:])

            def wr(p0, p1, c0, c1):  # bf16 slice of the bulk weight blob
                return wbr[p0:p1, c0 - c1w:c1 - c1w]

            b1col = wb32[0:128, C_B1:C_B1 + 1].bitcast(F32)
            b2col = [wb32[0:100, C_B2[s]:C_B2[s] + 1].bitcast(F32)
                     for s in range(2)]
            bf1col = wb32[0:50, C_BF1:C_BF1 + 1].bitcast(F32)
            bf2col = wb32[0:10, C_BF2:C_BF2 + 1].bitcast(F32)

            # per-N-tile stage emitters -----------------------------------
            # all input DMAs ride the sync (SP) HWDGE queue: each dma_start
            # costs ~0.6us of issuing-engine time, which must not come out
            # of the ACT engine's budget (it stalls the pooling chain)
            def issue_rhs(nt, t, h):
                rhs = sb.tile([128, N], BF16, tag="rhs1",
                              bufs=cfg["rhs_bufs"])
                k = t * 2 + h
                nc.sync.dma_start(rhs[:],
                                  xt[nt:nt + 1, :, k * N:(k + 1) * N])
                return rhs

            def issue_rhs_big(nt):
                # one DMA for a whole N-tile's 12 conv1 blocks (nt>=1 only:
                # its coarse completion sem would stall tile 0's startup)
                big = sb.tile([128, 12 * N], BF16, tag="rhsbig", bufs=1,
                              name=f"rhsbig_{nt}")
                nc.sync.dma_start(big[:], xt[nt:nt + 1, :, :])
                return big

            def conv1_stage(nt, rhs_pre=None, big_pre=None):
                """conv1 + 2x2 maxpool (bias+relu fused). Pool results land
                in r2all block (q, h) with partitions (row-in-pair, col, ch);
                each half is then DMA-copied (partition shift only) into
                rowall so conv2 sees full-width per-row tiles
                [120 = (col12, ch), N]."""
                r2all = sb.tile([128, 12, N], BF16, tag="r2all", bufs=2,
                                name=f"r2all_{nt}")
                rowall = sb.tile([120, 12, N], BF16, tag="rowall", bufs=2,
                                 name=f"rowall_{nt}")
                big = big_pre
                ei = 0
                ri = 0
                for t in range(6):
                    for h in range(2):
                        if rhs_pre is not None:
                            rhs = rhs_pre[t, h]
                        else:
                            rhs = big[:, (t * 2 + h) * N:(t * 2 + h + 1) * N]
                        V = []
                        act_heavy = (nt == 0)
                        for j2 in range(2):
                            pa = ps.tile([128, N], F32, tag="p1e",
                                         bufs=cfg["p1_bufs"],
                                         name=f"p1e_{nt}_{t}_{h}_{j2}")
                            pb = ps.tile([128, N], F32, tag="p1o",
                                         bufs=cfg["p1_bufs"],
                                         name=f"p1o_{nt}_{t}_{h}_{j2}")
                            for par, pt in ((0, pa), (1, pb)):
                                co = C_LHST1[j2][par]
                                nc.tensor.matmul(pt[:],
                                                 wbc1[:, co:co + 128],
                                                 rhs[:], start=True, stop=True)
                            # u = relu(Pa + b): the relu makes the final
                            # plain max-fold equal relu(pool+b)
                            u = sb.tile([128, N], BF16, tag="u1", bufs=3)
                            v = sb.tile([128, N], BF16, tag=f"v1_{j2}", bufs=2)
                            if act_heavy and j2 == 1:
                                # both PSUM reads on ACT, bf16 max on DVE
                                u2 = sb.tile([128, N], BF16, tag="u2", bufs=2)
                                nc.scalar.activation(u[:], pa[:], Relu,
                                                     bias=b1col)
                                nc.scalar.activation(u2[:], pb[:], Relu,
                                                     bias=b1col)
                                nc.vector.tensor_max(v[:], u[:], u2[:])
                            else:
                                if (ei % cfg["cme_dve_mod"] == cfg["cme_dve_mod"] - 1
                                        or (nt == 0 and ei == 0)):
                                    nc.vector.tensor_scalar(u[:], pa[:], b1col,
                                                            0.0, Add, Max)
                                else:
                                    nc.scalar.activation(u[:], pa[:], Relu,
                                                         bias=b1col)
                                # v = max(Pb + b, u): column pool (DVE, 1 PSUM)
                                nc.vector.scalar_tensor_tensor(
                                    v[:], pb[:], b1col, u[:], Add, Max)
                            V.append(v)
                            ei += 1
                        # row pool (relu already folded into u): equal-base
                        k = t * 2 + h
                        nc.vector.tensor_max(r2all[:, k, :], V[0][:], V[1][:])
                        ri += 1
                        # repack: pooled rows 2t (jp=0) and 2t+1 (jp=1),
                        # h-half -> partitions [60h:60h+60) of row tiles
                        for jp in range(2):
                            nc.sync.dma_start(
                                rowall[60 * h:60 * h + 60,
                                       2 * t + jp:2 * t + jp + 1, :],
                                r2all[64 * jp:64 * jp + 60, k:k + 1, :])
                return rowall

            def conv2_fc1_stage(nt, r2, tail_cb=()):
                """conv2 + relu + fc1 accumulation (fc1 one group behind so
                the PE never waits on the relu engine). tail_cb: parts of the
                PREVIOUS tile's softmax tail, emitted between conv2 groups so
                their PE ops never head-block the FIFO while waiting on the
                ACT chain."""
                pfc1 = ps.tile([50, N], F32, tag="pfc1", bufs=1,
                               name=f"pfc1_{nt}")
                pending = []  # (a2_tile, fc1_col) awaiting fc1 matmul
                gi = 0
                for yo in range(10):
                    for s in range(2):
                        gidx0 = 2 * yo + s
                        if gidx0 in (2, 5, 8) and len(tail_cb) > gidx0 // 3:
                            tail_cb[gidx0 // 3]()
                        p2 = ps.tile([100, N], F32, tag="p2", bufs=2,
                                     name=f"p2_{nt}_{yo}_{s}")
                        for dy in range(3):
                            co = C_LHST2[dy][s]
                            nc.tensor.matmul(
                                p2[:], wr(0, 120, co, co + 100),
                                r2[0:120, yo + dy, :],
                                start=(dy == 0), stop=(dy == 2))
                        a2 = sb.tile([100, N], BF16, tag="a2", bufs=3,
                                     name=f"a2_{nt}_{yo}_{s}")
                        gidx = 2 * yo + s
                        if (gidx < 16
                                and gidx % cfg["a2_dve_mod"] == cfg["a2_dve_mod"] - 1):
                            nc.vector.tensor_scalar(a2[:], p2[:], b2col[s],
                                                    0.0, Add, Max)
                        else:
                            nc.scalar.activation(a2[:], p2[:], Relu,
                                                 bias=b2col[s])
                        pending.append((a2, C_LHSTF1[yo][s]))
                        if len(pending) > 1:
                            pa2, pcf = pending.pop(0)
                            nc.tensor.matmul(
                                pfc1[:], wr(0, 100, pcf, pcf + 50),
                                pa2[:],
                                start=(gi == 0), stop=False)
                            gi += 1
                pa2, pcf = pending.pop(0)
                nc.tensor.matmul(pfc1[:], wr(0, 100, pcf, pcf + 50),
                                 pa2[:], start=False, stop=True)
                return pfc1

            Mult = mybir.AluOpType.mult

            def tail_parts(nt, pfc1):
                """fc2 + log_softmax + output DMA, as 3 separately emittable
                parts so each PE op only enters the FIFO once its ACT
                producer has had time to run. The Exp/Ln activation tables
                are preloaded via dummy ops so the 1.3us ACT_TABLE_LOADs
                hide under other work."""
                n0 = nt * N
                a3 = sb.tile([50, N], F32R, tag="a3", bufs=2,
                             name=f"a3_{nt}")
                zps = ps.tile([10, N], F32, tag="ptail", bufs=1,
                              name=f"zps_{nt}")
                z = sb.tile([10, N], F32, tag="z", bufs=2, name=f"z_{nt}")
                ez = sb.tile([10, N], F32R, tag="ez", bufs=2,
                             name=f"ez_{nt}")
                sps = ps.tile([1, N], F32, tag="ptail", bufs=1,
                              name=f"sps_{nt}")
                lse = sb.tile([1, N], F32R, tag="lse", bufs=2,
                              name=f"lse_{nt}")
                bps = ps.tile([10, N], F32, tag="ptail", bufs=1,
                              name=f"bps_{nt}")
                osb = sb.tile([10, N], F32, tag="osb", bufs=2,
                              name=f"osb_{nt}")

                def part1():
                    # dummy Exp first: its table load runs on ACT well
                    # before the real Exp needs it
                    dmye = sb.tile([1, 1], F32R, tag="dmy", bufs=4,
                                   name=f"dmye_{nt}")
                    nc.scalar.activation(
                        dmye[:], wb32[0:1, C_ONES_COL:C_ONES_COL + 1]
                        .bitcast(F32), Exp)
                    nc.scalar.activation(a3[:], pfc1[:], Relu, bias=bf1col)
                    nc.tensor.matmul(zps[:],
                                     wb32[0:50, C_LHSTF2:C_LHSTF2 + 10],
                                     a3[:], start=True, stop=True)
                    # z = zps*Etot + bias on DVE (keeps ACT for Exp/Ln)
                    nc.vector.tensor_scalar(z[:], zps[:], Etot,
                                            bf2col, Mult, Add)
                    nc.scalar.activation(ez[:], zps[:], Exp,
                                         bias=bf2col, scale=Etot)
                    # dummy Ln right after Exp: its table load hides under
                    # the following conv2 groups / ones-matmul
                    dmyl = sb.tile([1, 1], F32R, tag="dmy", bufs=4,
                                   name=f"dmyl_{nt}")
                    nc.scalar.activation(
                        dmyl[:], wb32[0:1, C_ONES_COL:C_ONES_COL + 1]
                        .bitcast(F32), Ln)

                def part2():
                    nc.tensor.matmul(sps[:],
                                     wb32[0:10, C_ONES_COL:C_ONES_COL + 1],
                                     ez[:], start=True, stop=True)
                    nc.scalar.activation(lse[:], sps[:], Ln)

                def part3():
                    nc.tensor.matmul(bps[:],
                                     wb32[0:1, C_ONES_ROW:C_ONES_ROW + 10],
                                     lse[:], start=True, stop=True)
                    nc.vector.tensor_sub(osb[:], z[:], bps[:])
                    nc.sync.dma_start(out[:, n0:n0 + N], osb[:])

                return [part1, part2, part3]

            # interleave N-tiles: tile k+1's conv1 is emitted before tile
            # k's conv2, and tile k's softmax tail is emitted in parts
            # between tile k+1's conv2 groups
            for _rep in range(repeat):
                # tile 0's rhs DMAs go out before the bulk of the weight
                # blob so the scalar HWDGE queue serves them first
                rhs0 = {(t, h): issue_rhs(0, t, h)
                        for t in range(6) for h in range(2)}
                nc.scalar.dma_start(wbr[:], wb16d[:, c1w:W16_COLS])
                bigs = {nt: issue_rhs_big(nt) for nt in range(1, n_tiles)}
                r2s = {}
                for nt in range(n_tiles):
                    r2s[nt] = conv1_stage(nt, rhs_pre=rhs0 if nt == 0 else None,
                                          big_pre=bigs.get(nt))
                prev_tail = ()
                for nt in range(n_tiles):
                    pfc1 = conv2_fc1_stage(nt, r2s.pop(nt), tail_cb=prev_tail)
                    prev_tail = tail_parts(nt, pfc1)
                for p in prev_tail:
                    p()
    nc.compile()
    return nc


def _host_tiles(x):
    """Pre-tile the input per core: [N_TILES*12, 128, N] bf16, one
    contiguous block per conv1 rhs DMA. Block (nt, t, h) holds
    x rows 4t..4t+7, cols 12h..12h+15, images nt*N..nt*N+N-1 with
    partitions = (row-in-window, col-in-window)."""
    per_core = []
    for i in range(N_CORES):
        xs = x[i * BC:(i + 1) * BC, 0]                  # [BC, 28, 28]
        xsT = np.ascontiguousarray(xs.transpose(1, 2, 0)).astype(NPBF16)
        tiles = np.empty((N_TILES, 6, 2, 128, N), NPBF16)
        for t in range(6):
            for h in range(2):
                blk = xsT[4 * t:4 * t + 8, 12 * h:12 * h + 16, :]
                blk = blk.reshape(128, N_TILES, N)
                tiles[:, t, h] = blk.transpose(1, 0, 2)
        # [nt, partition, (t,h)-block * N] so one contiguous DMA covers a
        # whole N-tile and per-block slices stay natural
        per_core.append(np.ascontiguousarray(
            tiles.transpose(0, 3, 1, 2, 4).reshape(N_TILES, 128, 12 * N)))
    return per_core


def kernel(**inputs):
    global LAST_EXEC_TIME_NS, LAST_RESULTS
    x = np.ascontiguousarray(np.asarray(inputs["x"], dtype=np.float32))
    wb16, wb32, scales = _host_prep(
        {k: np.asarray(v) for k, v in inputs.items()})

    nc = build_program(scales)

    xtiles = _host_tiles(x)
    in_maps = [{"xt": xtiles[i], "wb16": wb16, "wb32": wb32}
               for i in range(N_CORES)]

    trace = bool(os.environ.get("KERNEL_TRACE"))
    res = run_bass_kernel_spmd(nc, in_maps, list(range(N_CORES)), trace=trace)
    LAST_EXEC_TIME_NS = res.exec_time_ns
    LAST_RESULTS = res

    out = np.empty((B_TOTAL, 10), np.float32)
    for i in range(N_CORES):
        out[i * BC:(i + 1) * BC] = res.results[i]["out"].T
    return out


# revision 24
# speedup vs baseline: 1.1021x; 1.1021x over previous
"""Trainium2 Bass kernel for the binarized ConvNet (nn_ConvNet_81501299409071).

Data-parallel over batch: 8192 images -> 8 NeuronCores x 1024 images.

Device pipeline (feature-major: features on partitions, batch on free dim),
everything is a matmul against exactly-representable +-1 Toeplitz weight
matrices; the DoReFa binarization scale E is folded into the post-matmul
activation ops (relu(acc*E + b)).

The conv/fc matmul path runs in bf16 (the +-1 weights are exact in bf16;
images round at ~0.4% rel which is far inside the 2e-2 gate). This halves
HBM traffic and doubles DVE throughput on the SBUF-side pooling ops. The
softmax tail stays fp32/f32r.

  conv1 5x5 (1->10ch):  6x2 input tiles [128=(8 rows x 16 cols), N=512],
      4 matmuls of M=120 per tile; M packed as (row-in-pair, ch, col-pair)
      so 2x2 maxpool is two full-width tensor_max ops whose outputs land
      directly in conv2-ready [120=(half, ch, col), N] tiles.
  conv2 3x3 (10->20ch): per output row, 3 accumulating K=120 matmuls x 2
      output-channel halves.
  fc1 2000->50: 20 accumulating K=100 matmuls (one per conv2 relu tile).
  fc2 50->10 + log_softmax: exp/ln on ACT, partition sum / broadcast via
      tiny ones-matmuls, final subtract on DVE.

Startup: the PE is kept busy with tiny self-dependent warm-up matmuls while
the first input/weight DMAs land, so the HAM clock gate is already at 8/8
when real matmuls start. Input is pre-tiled on the host so every rhs DMA is
a single contiguous block, issued alternately on both HWDGE queues.
"""
import os
import numpy as np
import ml_dtypes

import concourse.bass as bass
import concourse.tile as tile
from concourse import bacc, mybir
from concourse.bass_utils import run_bass_kernel_spmd

F32 = mybir.dt.float32
F32R = mybir.dt.float32r
BF16 = mybir.dt.bfloat16
NPBF16 = ml_dtypes.bfloat16

N_CORES = 8
B_TOTAL = 8192
BC = B_TOTAL // N_CORES  # 1024 images per core
N = 512                  # batch tile (free dim / PSUM bank)
N_TILES = BC // N

LAST_EXEC_TIME_NS = None
LAST_RESULTS = None

# ---------------------------------------------------------------------------
# weight blobs: wb16 (bf16) holds every matmul lhsT for conv1/conv2/fc1;
# wb32 (f32r) holds the fp32 tail operands + bias columns.
# ---------------------------------------------------------------------------
_off16 = 0
def _t16(n):
    global _off16
    c = _off16
    _off16 += n
    return c

_off32 = 0
def _t32(n):
    global _off32
    c = _off32
    _off32 += n
    return c

C_LHST1 = [[_t16(128) for _par in range(2)] for _jp in range(2)]    # [j2][par]
C_LHST2 = [[_t16(100) for _s in range(2)] for _dy in range(3)]      # [dy][s]
C_LHSTF1 = [[_t16(50) for _s in range(2)] for _yo in range(10)]     # [yo][s]
W16_COLS = _off16

C_LHSTF2 = _t32(10)           # K=50
C_ONES_ROW = _t32(10)         # [1,10] ones (broadcast lhsT)
C_ONES_COL = _t32(1)          # [10,1] ones (partition-sum lhsT)
C_B1 = _t32(1)                # [120,1]
C_B2 = [_t32(1) for _s in range(2)]  # [100,1] each
C_BF1 = _t32(1)               # [50,1]
C_BF2 = _t32(1)               # [10,1]
W32_COLS = _off32


def _host_prep(inputs):
    """Binarize weights, build +-1 Toeplitz matrices + bias columns packed
    into the two weight blobs, and the E scales."""
    w1, b1 = inputs["w1"], inputs["b1"]
    w2, b2 = inputs["w2"], inputs["b2"]
    fw1, fb1 = inputs["fw1"], inputs["fb1"]
    fw2, fb2 = inputs["fw2"], inputs["fb2"]

    scales = {
        "E1": float(np.mean(np.abs(w1))),
        "E2": float(np.mean(np.abs(w2))),
        "Ef1": float(np.mean(np.abs(fw1))),
        "Ef2": float(np.mean(np.abs(fw2))),
    }
    s1 = np.sign(w1).astype(np.float32)
    s2 = np.sign(w2).astype(np.float32)
    sf1 = np.sign(fw1).astype(np.float32)
    sf2 = np.sign(fw2).astype(np.float32)

    wb16 = np.zeros((128, W16_COLS), np.float32)
    wb32 = np.zeros((128, W32_COLS), np.float32)

    # conv1 Toeplitz [j2][par] (j2 = row-in-pair): M m = jp*64 + c*10 + oc
    # ((c, oc) col-major within each jp half so the pooled halves are
    # partition-contiguous and the conv2 row repack is a plain
    # partition-shifted DMA copy; pads zero)
    for j2 in range(2):
        for par in range(2):
            blk = np.zeros((128, 128), np.float32)
            for jp in range(2):
                j = 2 * jp + j2
                for oc in range(10):
                    for c in range(6):
                        m = jp * 64 + c * 10 + oc
                        xo = 2 * c + par
                        for dy in range(5):
                            r = j + dy
                            for dx in range(5):
                                xi = xo + dx
                                blk[r * 16 + xi, m] = s1[oc, 0, dy, dx]
            co = C_LHST1[j2][par]
            wb16[:, co:co + 128] = blk

    # conv2 Toeplitz [dy][s]: rhs is a full-width pooled row
    # K k = xi*10 + ci (12 cols x 10 ch); M m = oci*10 + xo
    for dy in range(3):
        for s_ in range(2):
            blk = np.zeros((120, 100), np.float32)
            for xi in range(12):
                for ci in range(10):
                    for oci in range(10):
                        for xo in range(10):
                            dx = xi - xo
                            if 0 <= dx < 3:
                                blk[xi * 10 + ci, oci * 10 + xo] = \
                                    s2[10 * s_ + oci, ci, dy, dx]
            co = C_LHST2[dy][s_]
            wb16[0:120, co:co + 100] = blk

    # fc1 [yo][s]: K p = oci*10+xo -> f = (10s+oci)*100 + yo*10 + xo
    for yo in range(10):
        for s in range(2):
            blk = np.zeros((100, 50), np.float32)
            for oci in range(10):
                for xo in range(10):
                    f = (10 * s + oci) * 100 + yo * 10 + xo
                    blk[oci * 10 + xo, :] = sf1[:, f]
            co = C_LHSTF1[yo][s]
            wb16[0:100, co:co + 50] = blk

    wb32[0:50, C_LHSTF2:C_LHSTF2 + 10] = sf2.T
    wb32[0, C_ONES_ROW:C_ONES_ROW + 10] = 1.0
    wb32[0:10, C_ONES_COL] = 1.0

    # bias columns, pre-divided by the accumulated binarization scales so
    # every bias+relu runs unscaled (relu(acc + b')) on any engine; the one
    # true scale Etot is applied at the logits.
    E1, E2, Ef1 = scales["E1"], scales["E2"], scales["Ef1"]
    b1v = np.zeros(128, np.float32)
    for jp in range(2):
        for c in range(6):
            for oc in range(10):
                b1v[jp * 64 + c * 10 + oc] = b1[oc] / E1
    wb32[:, C_B1] = b1v
    for s in range(2):
        b2v = np.repeat(b2[10 * s:10 * s + 10], 10).astype(np.float32)
        wb32[0:100, C_B2[s]] = b2v / (E1 * E2)
    wb32[0:50, C_BF1] = fb1 / (E1 * E2 * Ef1)
    wb32[0:10, C_BF2] = fb2
    return wb16.astype(NPBF16), wb32, scales


# tuning knobs (engine splits / pool sizing), overridable for sweeps
CFG = {
    "rowmax_gp_mod": 2,     # rowmax i -> gpsimd when i % mod < thr
    "rowmax_gp_thr": 0,     # (gpsimd TENSOR_TENSOR rejected by trn2 ISA)
    "cme_dve_mod": 6,       # u-extract i -> DVE when i % mod == mod-1, else ACT
    "a2_dve_mod": 4,        # a2 relu -> DVE when (2*yo+s) % mod == mod-1
    "p1_bufs": 2,
    "rhs_bufs": 16,
    "n_warm": 24,           # PE warm-up matmuls issued during startup DMA wait
}


def build_program(scales, n_tiles=N_TILES, bc=BC, cfg=None, repeat=1):
    """Build the single-core SPMD bass program."""
    cfg = {**CFG, **(cfg or {})}
    Etot = scales["E1"] * scales["E2"] * scales["Ef1"] * scales["Ef2"]
    Relu = mybir.ActivationFunctionType.Relu
    Ident = mybir.ActivationFunctionType.Identity
    Exp = mybir.ActivationFunctionType.Exp
    Ln = mybir.ActivationFunctionType.Ln
    Add = mybir.AluOpType.add
    Max = mybir.AluOpType.max

    nc = bacc.Bacc("TRN2", target_bir_lowering=False, debug=False)
    # pre-tiled input: one contiguous [128, N] block per (nt, t, h)
    xt = nc.dram_tensor("xt", [n_tiles, 128, 12 * N], BF16,
                        kind="ExternalInput").ap()
    wb16d = nc.dram_tensor("wb16", [128, W16_COLS], BF16,
                           kind="ExternalInput").ap()
    wb32d = nc.dram_tensor("wb32", [128, W32_COLS], F32R,
                           kind="ExternalInput").ap()
    out = nc.dram_tensor("out", [10, bc], F32, kind="ExternalOutput").ap()

    with tile.TileContext(nc) as tc:
        with tc.tile_pool(name="wpool", bufs=1) as wpool, \
             tc.tile_pool(name="sb", bufs=1) as sb, \
             tc.tile_pool(name="ps", bufs=1, space="PSUM") as ps:

            # --- PE warm-up: tiny self-contained matmuls with no DMA deps
            # keep the PE busy from the first instruction, so the HAM clock
            # gate is released (~3.4us of activity) before real work lands.
            # N=128 each so ~40 of them bridge the ~5us startup DMA window.
            warm = sb.tile([128, 128], BF16, tag="warm")
            nc.vector.memset(warm[:], 1.0)
            wps = ps.tile([8, 128], F32, tag="p2", bufs=2)
            for _k in range(cfg["n_warm"]):
                nc.tensor.matmul(wps[:], warm[:, 0:8], warm[:],
                                 start=True, stop=True)

            c1w = 4 * 128
            wbc1 = wpool.tile([128, c1w], BF16, tag="wbc1")
            wbr = wpool.tile([128, W16_COLS - c1w], BF16, tag="wbr")
            wb32 = wpool.tile([128, W32_COLS], F32R, tag="wb32")
            # conv1 weights in their own tile so its LDWs only wait on this
            # small DMA; the rest is issued AFTER tile 0's rhs DMAs (below)
            # so it doesn't delay them on the scalar HWDGE queue.
            nc.scalar.dma_start(wbc1[:], wb16d[:, 0:c1w])
            nc.scalar.dma_start(wb32[:], wb32d[:])

            def wr(p0, p1, c0, c1):  # bf16 slice of the bulk weight blob
                return wbr[p0:p1, c0 - c1w:c1 - c1w]

            b1col = wb32[0:128, C_B1:C_B1 + 1].bitcast(F32)
            b2col = [wb32[0:100, C_B2[s]:C_B2[s] + 1].bitcast(F32)
                     for s in range(2)]
            bf1col = wb32[0:50, C_BF1:C_BF1 + 1].bitcast(F32)
            bf2col = wb32[0:10, C_BF2:C_BF2 + 1].bitcast(F32)

            # per-N-tile stage emitters -----------------------------------
            # all input DMAs ride the sync (SP) HWDGE queue: each dma_start
            # costs ~0.6us of issuing-engine time, which must not come out
            # of the ACT engine's budget (it stalls the pooling chain)
            def issue_rhs(nt, t, h):
                rhs = sb.tile([128, N], BF16, tag="rhs1",
                              bufs=cfg["rhs_bufs"])
                k = t * 2 + h
                nc.sync.dma_start(rhs[:],
                                  xt[nt:nt + 1, :, k * N:(k + 1) * N])
                return rhs

            def issue_rhs_big(nt):
                # one DMA for a whole N-tile's 12 conv1 blocks (nt>=1 only:
                # its coarse completion sem would stall tile 0's startup)
                big = sb.tile([128, 12 * N], BF16, tag="rhsbig", bufs=1,
                              name=f"rhsbig_{nt}")
                nc.scalar.dma_start(big[:], xt[nt:nt + 1, :, :])
                return big

            def conv1_stage(nt, rhs_pre=None, big_pre=None):
                """conv1 + 2x2 maxpool (bias+relu fused). Pool results land
                in r2all block (q, h) with partitions (row-in-pair, col, ch);
                each half is then DMA-copied (partition shift only) into
                rowall so conv2 sees full-width per-row tiles
                [120 = (col12, ch), N]."""
                r2all = sb.tile([128, 12, N], BF16, tag="r2all", bufs=2,
                                name=f"r2all_{nt}")
                rowall = sb.tile([120, 12, N], BF16, tag="rowall", bufs=2,
                                 name=f"rowall_{nt}")
                big = big_pre
                ei = 0
                ri = 0
                for t in range(6):
                    for h in range(2):
                        if rhs_pre is not None:
                            rhs = rhs_pre[t, h]
                        else:
                            rhs = big[:, (t * 2 + h) * N:(t * 2 + h + 1) * N]
                        V = []
                        for j2 in range(2):
                            pa = ps.tile([128, N], F32, tag="p1e",
                                         bufs=cfg["p1_bufs"],
                                         name=f"p1e_{nt}_{t}_{h}_{j2}")
                            pb = ps.tile([128, N], F32, tag="p1o",
                                         bufs=cfg["p1_bufs"],
                                         name=f"p1o_{nt}_{t}_{h}_{j2}")
                            for par, pt in ((0, pa), (1, pb)):
                                co = C_LHST1[j2][par]
                                nc.tensor.matmul(pt[:],
                                                 wbc1[:, co:co + 128],
                                                 rhs[:], start=True, stop=True)
                            # u = relu(Pa + b): the relu makes the final
                            # plain max-fold equal relu(pool+b)
                            u = sb.tile([128, N], BF16, tag="u1", bufs=3)
                            v = sb.tile([128, N], BF16, tag=f"v1_{j2}", bufs=2)
                            if (ei % cfg["cme_dve_mod"] == cfg["cme_dve_mod"] - 1
                                    or (nt == 0 and ei == 0)):
                                nc.vector.tensor_scalar(u[:], pa[:], b1col,
                                                        0.0, Add, Max)
                            else:
                                nc.scalar.activation(u[:], pa[:], Relu,
                                                     bias=b1col)
                            # v = max(Pb + b, u): column pool (DVE, 1 PSUM)
                            nc.vector.scalar_tensor_tensor(
                                v[:], pb[:], b1col, u[:], Add, Max)
                            V.append(v)
                            ei += 1
                        # row pool (relu already folded into u): equal-base
                        k = t * 2 + h
                        nc.vector.tensor_max(r2all[:, k, :], V[0][:], V[1][:])
                        ri += 1
                        # repack: pooled rows 2t (jp=0) and 2t+1 (jp=1),
                        # h-half -> partitions [60h:60h+60) of row tiles
                        for jp in range(2):
                            nc.sync.dma_start(
                                rowall[60 * h:60 * h + 60,
                                       2 * t + jp:2 * t + jp + 1, :],
                                r2all[64 * jp:64 * jp + 60, k:k + 1, :])
                return rowall

            def conv2_fc1_stage(nt, r2, tail_cb=()):
                """conv2 + relu + fc1 accumulation (fc1 one group behind so
                the PE never waits on the relu engine). tail_cb: parts of the
                PREVIOUS tile's softmax tail, emitted between conv2 groups so
                their PE ops never head-block the FIFO while waiting on the
                ACT chain."""
                pfc1 = ps.tile([50, N], F32, tag="pfc1", bufs=1,
                               name=f"pfc1_{nt}")
                pending = []  # (a2_tile, fc1_col) awaiting fc1 matmul
                gi = 0
                for yo in range(10):
                    for s in range(2):
                        gidx0 = 2 * yo + s
                        if gidx0 in (2, 5, 8) and len(tail_cb) > gidx0 // 3:
                            tail_cb[gidx0 // 3]()
                        p2 = ps.tile([100, N], F32, tag="p2", bufs=2,
                                     name=f"p2_{nt}_{yo}_{s}")
                        for dy in range(3):
                            co = C_LHST2[dy][s]
                            nc.tensor.matmul(
                                p2[:], wr(0, 120, co, co + 100),
                                r2[0:120, yo + dy, :],
                                start=(dy == 0), stop=(dy == 2))
                        a2 = sb.tile([100, N], BF16, tag="a2", bufs=3,
                                     name=f"a2_{nt}_{yo}_{s}")
                        gidx = 2 * yo + s
                        if (gidx < 16
                                and gidx % cfg["a2_dve_mod"] == cfg["a2_dve_mod"] - 1):
                            nc.vector.tensor_scalar(a2[:], p2[:], b2col[s],
                                                    0.0, Add, Max)
                        else:
                            nc.scalar.activation(a2[:], p2[:], Relu,
                                                 bias=b2col[s])
                        pending.append((a2, C_LHSTF1[yo][s]))
                        if len(pending) > 1:
                            pa2, pcf = pending.pop(0)
                            nc.tensor.matmul(
                                pfc1[:], wr(0, 100, pcf, pcf + 50),
                                pa2[:],
                                start=(gi == 0), stop=False)
                            gi += 1
                pa2, pcf = pending.pop(0)
                nc.tensor.matmul(pfc1[:], wr(0, 100, pcf, pcf + 50),
                                 pa2[:], start=False, stop=True)
                return pfc1

            Mult = mybir.AluOpType.mult

            def tail_parts(nt, pfc1):
                """fc2 + log_softmax + output DMA, as 3 separately emittable
                parts so each PE op only enters the FIFO once its ACT
                producer has had time to run. The Exp/Ln activation tables
                are preloaded via dummy ops so the 1.3us ACT_TABLE_LOADs
                hide under other work."""
                n0 = nt * N
                a3 = sb.tile([50, N], F32R, tag="a3", bufs=2,
                             name=f"a3_{nt}")
                zps = ps.tile([10, N], F32, tag="ptail", bufs=1,
                              name=f"zps_{nt}")
                z = sb.tile([10, N], F32, tag="z", bufs=2, name=f"z_{nt}")
                ez = sb.tile([10, N], F32R, tag="ez", bufs=2,
                             name=f"ez_{nt}")
                sps = ps.tile([1, N], F32, tag="ptail", bufs=1,
                              name=f"sps_{nt}")
                lse = sb.tile([1, N], F32R, tag="lse", bufs=2,
                              name=f"lse_{nt}")
                bps = ps.tile([10, N], F32, tag="ptail", bufs=1,
                              name=f"bps_{nt}")
                osb = sb.tile([10, N], F32, tag="osb", bufs=2,
                              name=f"osb_{nt}")

                def part1():
                    # dummy Exp first: its table load runs on ACT well
                    # before the real Exp needs it
                    dmye = sb.tile([1, 1], F32R, tag="dmy", bufs=4,
                                   name=f"dmye_{nt}")
                    nc.scalar.activation(
                        dmye[:], wb32[0:1, C_ONES_COL:C_ONES_COL + 1]
                        .bitcast(F32), Exp)
                    nc.scalar.activation(a3[:], pfc1[:], Relu, bias=bf1col)
                    nc.tensor.matmul(zps[:],
                                     wb32[0:50, C_LHSTF2:C_LHSTF2 + 10],
                                     a3[:], start=True, stop=True)
                    # z = zps*Etot + bias on DVE (keeps ACT for Exp/Ln)
                    nc.vector.tensor_scalar(z[:], zps[:], Etot,
                                            bf2col, Mult, Add)
                    nc.scalar.activation(ez[:], zps[:], Exp,
                                         bias=bf2col, scale=Etot)
                    # dummy Ln right after Exp: its table load hides under
                    # the following conv2 groups / ones-matmul
                    dmyl = sb.tile([1, 1], F32R, tag="dmy", bufs=4,
                                   name=f"dmyl_{nt}")
                    nc.scalar.activation(
                        dmyl[:], wb32[0:1, C_ONES_COL:C_ONES_COL + 1]
                        .bitcast(F32), Ln)

                def part2():
                    nc.tensor.matmul(sps[:],
                                     wb32[0:10, C_ONES_COL:C_ONES_COL + 1],
                                     ez[:], start=True, stop=True)
                    nc.scalar.activation(lse[:], sps[:], Ln)

                def part3():
                    nc.tensor.matmul(bps[:],
                                     wb32[0:1, C_ONES_ROW:C_ONES_ROW + 10],
                                     lse[:], start=True, stop=True)
                    nc.vector.tensor_sub(osb[:], z[:], bps[:])
                    nc.sync.dma_start(out[:, n0:n0 + N], osb[:])

                return [part1, part2, part3]

            # interleave N-tiles: tile k+1's conv1 is emitted before tile
            # k's conv2, and tile k's softmax tail is emitted in parts
            # between tile k+1's conv2 groups
            for _rep in range(repeat):
                # tile 0's rhs DMAs go out before the bulk of the weight
                # blob so the scalar HWDGE queue serves them first
                rhs0 = {(t, h): issue_rhs(0, t, h)
                        for t in range(6) for h in range(2)}
                nc.scalar.dma_start(wbr[:], wb16d[:, c1w:W16_COLS])
                bigs = {nt: issue_rhs_big(nt) for nt in range(1, n_tiles)}
                r2s = {}
                for nt in range(n_tiles):
                    r2s[nt] = conv1_stage(nt, rhs_pre=rhs0 if nt == 0 else None,
                                          big_pre=bigs.get(nt))
                prev_tail = ()
                for nt in range(n_tiles):
                    pfc1 = conv2_fc1_stage(nt, r2s.pop(nt), tail_cb=prev_tail)
                    prev_tail = tail_parts(nt, pfc1)
                for p in prev_tail:
                    p()
    nc.compile()
    return nc


def _host_tiles(x):
    """Pre-tile the input per core: [N_TILES*12, 128, N] bf16, one
    contiguous block per conv1 rhs DMA. Block (nt, t, h) holds
    x rows 4t..4t+7, cols 12h..12h+15, images nt*N..nt*N+N-1 with
    partitions = (row-in-window, col-in-window)."""
    per_core = []
    for i in range(N_CORES):
        xs = x[i * BC:(i + 1) * BC, 0]                  # [BC, 28, 28]
        xsT = np.ascontiguousarray(xs.transpose(1, 2, 0)).astype(NPBF16)
        tiles = np.empty((N_TILES, 6, 2, 128, N), NPBF16)
        for t in range(6):
            for h in range(2):
                blk = xsT[4 * t:4 * t + 8, 12 * h:12 * h + 16, :]
                blk = blk.reshape(128, N_TILES, N)
                tiles[:, t, h] = blk.transpose(1, 0, 2)
        # [nt, partition, (t,h)-block * N] so one contiguous DMA covers a
        # whole N-tile and per-block slices stay natural
        per_core.append(np.ascontiguousarray(
            tiles.transpose(0, 3, 1, 2, 4).reshape(N_TILES, 128, 12 * N)))
    return per_core


def kernel(**inputs):
    global LAST_EXEC_TIME_NS, LAST_RESULTS
    x = np.ascontiguousarray(np.asarray(inputs["x"], dtype=np.float32))
    wb16, wb32, scales = _host_prep(
        {k: np.asarray(v) for k, v in inputs.items()})

    nc = build_program(scales)

    xtiles = _host_tiles(x)
    in_maps = [{"xt": xtiles[i], "wb16": wb16, "wb32": wb32}
               for i in range(N_CORES)]

    trace = bool(os.environ.get("KERNEL_TRACE"))
    res = run_bass_kernel_spmd(nc, in_maps, list(range(N_CORES)), trace=trace)
    LAST_EXEC_TIME_NS = res.exec_time_ns
    LAST_RESULTS = res

    out = np.empty((B_TOTAL, 10), np.float32)
    for i in range(N_CORES):
        out[i * BC:(i + 1) * BC] = res.results[i]["out"].T
    return out


# revision 25
# speedup vs baseline: 1.1252x; 1.0210x over previous
"""Trainium2 Bass kernel for the binarized ConvNet (nn_ConvNet_81501299409071).

Data-parallel over batch: 8192 images -> 8 NeuronCores x 1024 images.

Device pipeline (feature-major: features on partitions, batch on free dim),
everything is a matmul against exactly-representable +-1 Toeplitz weight
matrices; the DoReFa binarization scale E is folded into the post-matmul
activation ops (relu(acc*E + b)).

The conv/fc matmul path runs in bf16 (the +-1 weights are exact in bf16;
images round at ~0.4% rel which is far inside the 2e-2 gate). This halves
HBM traffic and doubles DVE throughput on the SBUF-side pooling ops. The
softmax tail stays fp32/f32r.

  conv1 5x5 (1->10ch):  6x2 input tiles [128=(8 rows x 16 cols), N=512],
      4 matmuls of M=120 per tile; M packed as (row-in-pair, ch, col-pair)
      so 2x2 maxpool is two full-width tensor_max ops whose outputs land
      directly in conv2-ready [120=(half, ch, col), N] tiles.
  conv2 3x3 (10->20ch): per output row, 3 accumulating K=120 matmuls x 2
      output-channel halves.
  fc1 2000->50: 20 accumulating K=100 matmuls (one per conv2 relu tile).
  fc2 50->10 + log_softmax: exp/ln on ACT, partition sum / broadcast via
      tiny ones-matmuls, final subtract on DVE.

Startup: the PE is kept busy with tiny self-dependent warm-up matmuls while
the first input/weight DMAs land, so the HAM clock gate is already at 8/8
when real matmuls start. Input is pre-tiled on the host so every rhs DMA is
a single contiguous block, issued alternately on both HWDGE queues.
"""
import os
import numpy as np
import ml_dtypes

import concourse.bass as bass
import concourse.tile as tile
from concourse import bacc, mybir
from concourse.bass_utils import run_bass_kernel_spmd

F32 = mybir.dt.float32
F32R = mybir.dt.float32r
BF16 = mybir.dt.bfloat16
NPBF16 = ml_dtypes.bfloat16

N_CORES = 8
B_TOTAL = 8192
BC = B_TOTAL // N_CORES  # 1024 images per core
N = 512                  # batch tile (free dim / PSUM bank)
N_TILES = BC // N

LAST_EXEC_TIME_NS = None
LAST_RESULTS = None

# ---------------------------------------------------------------------------
# weight blobs: wb16 (bf16) holds every matmul lhsT for conv1/conv2/fc1;
# wb32 (f32r) holds the fp32 tail operands + bias columns.
# ---------------------------------------------------------------------------
_off16 = 0
def _t16(n):
    global _off16
    c = _off16
    _off16 += n
    return c

_off32 = 0
def _t32(n):
    global _off32
    c = _off32
    _off32 += n
    return c

C_LHST1 = [[_t16(128) for _par in range(2)] for _jp in range(2)]    # [j2][par]
C_LHST2 = [[_t16(100) for _s in range(2)] for _dy in range(3)]      # [dy][s]
C_LHSTF1 = [[_t16(50) for _s in range(2)] for _yo in range(10)]     # [yo][s]
W16_COLS = _off16

C_LHSTF2 = _t32(10)           # K=50
C_ONES_ROW = _t32(10)         # [1,10] ones (broadcast lhsT)
C_ONES_COL = _t32(1)          # [10,1] ones (partition-sum lhsT)
C_B1 = _t32(1)                # [120,1]
C_B2 = [_t32(1) for _s in range(2)]  # [100,1] each
C_BF1 = _t32(1)               # [50,1]
C_BF2 = _t32(1)               # [10,1]
W32_COLS = _off32


def _host_prep(inputs):
    """Binarize weights, build +-1 Toeplitz matrices + bias columns packed
    into the two weight blobs, and the E scales."""
    w1, b1 = inputs["w1"], inputs["b1"]
    w2, b2 = inputs["w2"], inputs["b2"]
    fw1, fb1 = inputs["fw1"], inputs["fb1"]
    fw2, fb2 = inputs["fw2"], inputs["fb2"]

    scales = {
        "E1": float(np.mean(np.abs(w1))),
        "E2": float(np.mean(np.abs(w2))),
        "Ef1": float(np.mean(np.abs(fw1))),
        "Ef2": float(np.mean(np.abs(fw2))),
    }
    s1 = np.sign(w1).astype(np.float32)
    s2 = np.sign(w2).astype(np.float32)
    sf1 = np.sign(fw1).astype(np.float32)
    sf2 = np.sign(fw2).astype(np.float32)

    wb16 = np.zeros((128, W16_COLS), np.float32)
    wb32 = np.zeros((128, W32_COLS), np.float32)

    # conv1 Toeplitz [j2][par] (j2 = row-in-pair): M m = jp*64 + c*10 + oc
    # ((c, oc) col-major within each jp half so the pooled halves are
    # partition-contiguous and the conv2 row repack is a plain
    # partition-shifted DMA copy; pads zero)
    for j2 in range(2):
        for par in range(2):
            blk = np.zeros((128, 128), np.float32)
            for jp in range(2):
                j = 2 * jp + j2
                for oc in range(10):
                    for c in range(6):
                        m = jp * 64 + c * 10 + oc
                        xo = 2 * c + par
                        for dy in range(5):
                            r = j + dy
                            for dx in range(5):
                                xi = xo + dx
                                blk[r * 16 + xi, m] = s1[oc, 0, dy, dx]
            co = C_LHST1[j2][par]
            wb16[:, co:co + 128] = blk

    # conv2 Toeplitz [dy][s]: rhs is a full-width pooled row
    # K k = xi*10 + ci (12 cols x 10 ch); M m = oci*10 + xo
    for dy in range(3):
        for s_ in range(2):
            blk = np.zeros((120, 100), np.float32)
            for xi in range(12):
                for ci in range(10):
                    for oci in range(10):
                        for xo in range(10):
                            dx = xi - xo
                            if 0 <= dx < 3:
                                blk[xi * 10 + ci, oci * 10 + xo] = \
                                    s2[10 * s_ + oci, ci, dy, dx]
            co = C_LHST2[dy][s_]
            wb16[0:120, co:co + 100] = blk

    # fc1 [yo][s]: K p = oci*10+xo -> f = (10s+oci)*100 + yo*10 + xo
    for yo in range(10):
        for s in range(2):
            blk = np.zeros((100, 50), np.float32)
            for oci in range(10):
                for xo in range(10):
                    f = (10 * s + oci) * 100 + yo * 10 + xo
                    blk[oci * 10 + xo, :] = sf1[:, f]
            co = C_LHSTF1[yo][s]
            wb16[0:100, co:co + 50] = blk

    wb32[0:50, C_LHSTF2:C_LHSTF2 + 10] = sf2.T
    wb32[0, C_ONES_ROW:C_ONES_ROW + 10] = 1.0
    wb32[0:10, C_ONES_COL] = 1.0

    # bias columns, pre-divided by the accumulated binarization scales so
    # every bias+relu runs unscaled (relu(acc + b')) on any engine; the one
    # true scale Etot is applied at the logits.
    E1, E2, Ef1 = scales["E1"], scales["E2"], scales["Ef1"]
    b1v = np.zeros(128, np.float32)
    for jp in range(2):
        for c in range(6):
            for oc in range(10):
                b1v[jp * 64 + c * 10 + oc] = b1[oc] / E1
    wb32[:, C_B1] = b1v
    for s in range(2):
        b2v = np.repeat(b2[10 * s:10 * s + 10], 10).astype(np.float32)
        wb32[0:100, C_B2[s]] = b2v / (E1 * E2)
    wb32[0:50, C_BF1] = fb1 / (E1 * E2 * Ef1)
    wb32[0:10, C_BF2] = fb2
    return wb16.astype(NPBF16), wb32, scales


# tuning knobs (engine splits / pool sizing), overridable for sweeps
CFG = {
    "rowmax_gp_mod": 2,     # rowmax i -> gpsimd when i % mod < thr
    "rowmax_gp_thr": 0,     # (gpsimd TENSOR_TENSOR rejected by trn2 ISA)
    "cme_dve_mod": 6,       # u-extract i -> DVE when i % mod == mod-1, else ACT
    "a2_dve_mod": 4,        # a2 relu -> DVE when (2*yo+s) % mod == mod-1
    "p1_bufs": 2,
    "rhs_bufs": 16,
    "n_warm": 24,           # PE warm-up matmuls issued during startup DMA wait
}


def build_program(scales, n_tiles=N_TILES, bc=BC, cfg=None, repeat=1):
    """Build the single-core SPMD bass program."""
    cfg = {**CFG, **(cfg or {})}
    Etot = scales["E1"] * scales["E2"] * scales["Ef1"] * scales["Ef2"]
    Relu = mybir.ActivationFunctionType.Relu
    Ident = mybir.ActivationFunctionType.Identity
    Exp = mybir.ActivationFunctionType.Exp
    Ln = mybir.ActivationFunctionType.Ln
    Add = mybir.AluOpType.add
    Max = mybir.AluOpType.max

    nc = bacc.Bacc("TRN2", target_bir_lowering=False, debug=False)
    # pre-tiled input: one contiguous [128, N] block per (nt, t, h)
    xt = nc.dram_tensor("xt", [n_tiles, 128, 12 * N], BF16,
                        kind="ExternalInput").ap()
    wb16d = nc.dram_tensor("wb16", [128, W16_COLS], BF16,
                           kind="ExternalInput").ap()
    wb32d = nc.dram_tensor("wb32", [128, W32_COLS], F32R,
                           kind="ExternalInput").ap()
    out = nc.dram_tensor("out", [10, bc], F32, kind="ExternalOutput").ap()

    with tile.TileContext(nc) as tc:
        with tc.tile_pool(name="wpool", bufs=1) as wpool, \
             tc.tile_pool(name="sb", bufs=1) as sb, \
             tc.tile_pool(name="ps", bufs=1, space="PSUM") as ps:

            # --- PE warm-up: tiny self-contained matmuls with no DMA deps
            # keep the PE busy from the first instruction, so the HAM clock
            # gate is released (~3.4us of activity) before real work lands.
            # N=128 each so ~40 of them bridge the ~5us startup DMA window.
            warm = sb.tile([128, 128], BF16, tag="warm")
            nc.vector.memset(warm[:], 1.0)
            wps = ps.tile([8, 128], F32, tag="p2", bufs=2)
            for _k in range(cfg["n_warm"]):
                nc.tensor.matmul(wps[:], warm[:, 0:8], warm[:],
                                 start=True, stop=True)

            c1w = 4 * 128
            wbc1 = wpool.tile([128, c1w], BF16, tag="wbc1")
            wbr = wpool.tile([128, W16_COLS - c1w], BF16, tag="wbr")
            wb32 = wpool.tile([128, W32_COLS], F32R, tag="wb32")
            # conv1 weights in their own tile so its LDWs only wait on this
            # small DMA; the rest is issued AFTER tile 0's rhs DMAs (below)
            # so it doesn't delay them on the scalar HWDGE queue.
            nc.scalar.dma_start(wbc1[:], wb16d[:, 0:c1w])
            nc.scalar.dma_start(wb32[:], wb32d[:])

            def wr(p0, p1, c0, c1):  # bf16 slice of the bulk weight blob
                return wbr[p0:p1, c0 - c1w:c1 - c1w]

            b1col = wb32[0:128, C_B1:C_B1 + 1].bitcast(F32)
            b2col = [wb32[0:100, C_B2[s]:C_B2[s] + 1].bitcast(F32)
                     for s in range(2)]
            bf1col = wb32[0:50, C_BF1:C_BF1 + 1].bitcast(F32)
            bf2col = wb32[0:10, C_BF2:C_BF2 + 1].bitcast(F32)

            # per-N-tile stage emitters -----------------------------------
            # all input DMAs ride the sync (SP) HWDGE queue: each dma_start
            # costs ~0.6us of issuing-engine time, which must not come out
            # of the ACT engine's budget (it stalls the pooling chain)
            def issue_rhs(nt, t, h):
                rhs = sb.tile([128, N], BF16, tag="rhs1",
                              bufs=cfg["rhs_bufs"])
                k = t * 2 + h
                nc.sync.dma_start(rhs[:],
                                  xt[nt:nt + 1, :, k * N:(k + 1) * N])
                return rhs

            def issue_rhs_big(nt):
                # one DMA for a whole N-tile's 12 conv1 blocks (nt>=1 only:
                # its coarse completion sem would stall tile 0's startup)
                big = sb.tile([128, 12 * N], BF16, tag="rhsbig", bufs=1,
                              name=f"rhsbig_{nt}")
                nc.sync.dma_start(big[:], xt[nt:nt + 1, :, :])
                return big

            def conv1_stage(nt, rhs_pre=None):
                """conv1 + 2x2 maxpool (bias+relu fused). Pool results land
                in r2all block (q, h) with partitions (row-in-pair, col, ch);
                each half is then DMA-copied (partition shift only) into
                rowall so conv2 sees full-width per-row tiles
                [120 = (col12, ch), N]."""
                r2all = sb.tile([128, 12, N], BF16, tag="r2all", bufs=2,
                                name=f"r2all_{nt}")
                rowall = sb.tile([120, 12, N], BF16, tag="rowall", bufs=2,
                                 name=f"rowall_{nt}")
                big = None if rhs_pre is not None else issue_rhs_big(nt)
                ei = 0
                ri = 0
                for t in range(6):
                    for h in range(2):
                        if rhs_pre is not None:
                            rhs = rhs_pre[t, h]
                        else:
                            rhs = big[:, (t * 2 + h) * N:(t * 2 + h + 1) * N]
                        V = []
                        for j2 in range(2):
                            pa = ps.tile([128, N], F32, tag="p1e",
                                         bufs=cfg["p1_bufs"],
                                         name=f"p1e_{nt}_{t}_{h}_{j2}")
                            pb = ps.tile([128, N], F32, tag="p1o",
                                         bufs=cfg["p1_bufs"],
                                         name=f"p1o_{nt}_{t}_{h}_{j2}")
                            for par, pt in ((0, pa), (1, pb)):
                                co = C_LHST1[j2][par]
                                nc.tensor.matmul(pt[:],
                                                 wbc1[:, co:co + 128],
                                                 rhs[:], start=True, stop=True)
                            # u = relu(Pa + b): the relu makes the final
                            # plain max-fold equal relu(pool+b)
                            u = sb.tile([128, N], BF16, tag="u1", bufs=3)
                            v = sb.tile([128, N], BF16, tag=f"v1_{j2}", bufs=2)
                            if (ei % cfg["cme_dve_mod"] == cfg["cme_dve_mod"] - 1
                                    or (nt == 0 and ei == 0)):
                                nc.vector.tensor_scalar(u[:], pa[:], b1col,
                                                        0.0, Add, Max)
                            else:
                                nc.scalar.activation(u[:], pa[:], Relu,
                                                     bias=b1col)
                            # v = max(Pb + b, u): column pool (DVE, 1 PSUM)
                            nc.vector.scalar_tensor_tensor(
                                v[:], pb[:], b1col, u[:], Add, Max)
                            V.append(v)
                            ei += 1
                        # row pool (relu already folded into u): equal-base
                        k = t * 2 + h
                        nc.vector.tensor_max(r2all[:, k, :], V[0][:], V[1][:])
                        ri += 1
                        # repack: pooled rows 2t (jp=0) and 2t+1 (jp=1),
                        # h-half -> partitions [60h:60h+60) of row tiles
                        for jp in range(2):
                            nc.sync.dma_start(
                                rowall[60 * h:60 * h + 60,
                                       2 * t + jp:2 * t + jp + 1, :],
                                r2all[64 * jp:64 * jp + 60, k:k + 1, :])
                return rowall

            def conv2_fc1_stage(nt, r2, tail_cb=()):
                """conv2 + relu + fc1 accumulation (fc1 one group behind so
                the PE never waits on the relu engine). tail_cb: parts of the
                PREVIOUS tile's softmax tail, emitted between conv2 groups so
                their PE ops never head-block the FIFO while waiting on the
                ACT chain."""
                pfc1 = ps.tile([50, N], F32, tag="pfc1", bufs=1,
                               name=f"pfc1_{nt}")
                pending = []  # (a2_tile, fc1_col) awaiting fc1 matmul
                gi = 0
                for yo in range(10):
                    for s in range(2):
                        gidx0 = 2 * yo + s
                        if gidx0 in (2, 5, 8) and len(tail_cb) > gidx0 // 3:
                            tail_cb[gidx0 // 3]()
                        p2 = ps.tile([100, N], F32, tag="p2", bufs=2,
                                     name=f"p2_{nt}_{yo}_{s}")
                        for dy in range(3):
                            co = C_LHST2[dy][s]
                            nc.tensor.matmul(
                                p2[:], wr(0, 120, co, co + 100),
                                r2[0:120, yo + dy, :],
                                start=(dy == 0), stop=(dy == 2))
                        a2 = sb.tile([100, N], BF16, tag="a2", bufs=3,
                                     name=f"a2_{nt}_{yo}_{s}")
                        gidx = 2 * yo + s
                        if (gidx < 16
                                and gidx % cfg["a2_dve_mod"] == cfg["a2_dve_mod"] - 1):
                            nc.vector.tensor_scalar(a2[:], p2[:], b2col[s],
                                                    0.0, Add, Max)
                        else:
                            nc.scalar.activation(a2[:], p2[:], Relu,
                                                 bias=b2col[s])
                        pending.append((a2, C_LHSTF1[yo][s]))
                        if len(pending) > 1:
                            pa2, pcf = pending.pop(0)
                            nc.tensor.matmul(
                                pfc1[:], wr(0, 100, pcf, pcf + 50),
                                pa2[:],
                                start=(gi == 0), stop=False)
                            gi += 1
                pa2, pcf = pending.pop(0)
                nc.tensor.matmul(pfc1[:], wr(0, 100, pcf, pcf + 50),
                                 pa2[:], start=False, stop=True)
                return pfc1

            Mult = mybir.AluOpType.mult

            def tail_parts(nt, pfc1):
                """fc2 + log_softmax + output DMA, as 3 separately emittable
                parts so each PE op only enters the FIFO once its ACT
                producer has had time to run. The Exp/Ln activation tables
                are preloaded via dummy ops so the 1.3us ACT_TABLE_LOADs
                hide under other work."""
                n0 = nt * N
                a3 = sb.tile([50, N], F32R, tag="a3", bufs=2,
                             name=f"a3_{nt}")
                zps = ps.tile([10, N], F32, tag="ptail", bufs=1,
                              name=f"zps_{nt}")
                z = sb.tile([10, N], F32, tag="z", bufs=2, name=f"z_{nt}")
                ez = sb.tile([10, N], F32R, tag="ez", bufs=2,
                             name=f"ez_{nt}")
                sps = ps.tile([1, N], F32, tag="ptail", bufs=1,
                              name=f"sps_{nt}")
                lse = sb.tile([1, N], F32R, tag="lse", bufs=2,
                              name=f"lse_{nt}")
                bps = ps.tile([10, N], F32, tag="ptail", bufs=1,
                              name=f"bps_{nt}")
                osb = sb.tile([10, N], F32, tag="osb", bufs=2,
                              name=f"osb_{nt}")

                def part1():
                    # dummy Exp first: its table load runs on ACT well
                    # before the real Exp needs it
                    dmye = sb.tile([1, 1], F32R, tag="dmy", bufs=4,
                                   name=f"dmye_{nt}")
                    nc.scalar.activation(
                        dmye[:], wb32[0:1, C_ONES_COL:C_ONES_COL + 1]
                        .bitcast(F32), Exp)
                    nc.scalar.activation(a3[:], pfc1[:], Relu, bias=bf1col)
                    nc.tensor.matmul(zps[:],
                                     wb32[0:50, C_LHSTF2:C_LHSTF2 + 10],
                                     a3[:], start=True, stop=True)
                    # z = zps*Etot + bias on DVE (keeps ACT for Exp/Ln)
                    nc.vector.tensor_scalar(z[:], zps[:], Etot,
                                            bf2col, Mult, Add)
                    nc.scalar.activation(ez[:], zps[:], Exp,
                                         bias=bf2col, scale=Etot)
                    # dummy Ln right after Exp: its table load hides under
                    # the following conv2 groups / ones-matmul
                    dmyl = sb.tile([1, 1], F32R, tag="dmy", bufs=4,
                                   name=f"dmyl_{nt}")
                    nc.scalar.activation(
                        dmyl[:], wb32[0:1, C_ONES_COL:C_ONES_COL + 1]
                        .bitcast(F32), Ln)

                def part2():
                    nc.tensor.matmul(sps[:],
                                     wb32[0:10, C_ONES_COL:C_ONES_COL + 1],
                                     ez[:], start=True, stop=True)
                    nc.scalar.activation(lse[:], sps[:], Ln)

                def part3():
                    nc.tensor.matmul(bps[:],
                                     wb32[0:1, C_ONES_ROW:C_ONES_ROW + 10],
                                     lse[:], start=True, stop=True)
                    nc.vector.tensor_sub(osb[:], z[:], bps[:])
                    nc.sync.dma_start(out[:, n0:n0 + N], osb[:])

                return [part1, part2, part3]

            # interleave N-tiles: tile k+1's conv1 is emitted before tile
            # k's conv2, and tile k's softmax tail is emitted in parts
            # between tile k+1's conv2 groups
            for _rep in range(repeat):
                # tile 0's rhs DMAs go out before the bulk of the weight
                # blob so the scalar HWDGE queue serves them first
                rhs0 = {(t, h): issue_rhs(0, t, h)
                        for t in range(6) for h in range(2)}
                nc.scalar.dma_start(wbr[:], wb16d[:, c1w:W16_COLS])
                r2s = {}
                for nt in range(n_tiles):
                    r2s[nt] = conv1_stage(nt, rhs_pre=rhs0 if nt == 0 else None)
                prev_tail = ()
                for nt in range(n_tiles):
                    pfc1 = conv2_fc1_stage(nt, r2s.pop(nt), tail_cb=prev_tail)
                    prev_tail = tail_parts(nt, pfc1)
                for p in prev_tail:
                    p()
    nc.compile()
    return nc


def _host_tiles(x):
    """Pre-tile the input per core: [N_TILES*12, 128, N] bf16, one
    contiguous block per conv1 rhs DMA. Block (nt, t, h) holds
    x rows 4t..4t+7, cols 12h..12h+15, images nt*N..nt*N+N-1 with
    partitions = (row-in-window, col-in-window)."""
    per_core = []
    for i in range(N_CORES):
        xs = x[i * BC:(i + 1) * BC, 0]                  # [BC, 28, 28]
        xsT = np.ascontiguousarray(xs.transpose(1, 2, 0)).astype(NPBF16)
        tiles = np.empty((N_TILES, 6, 2, 128, N), NPBF16)
        for t in range(6):
            for h in range(2):
                blk = xsT[4 * t:4 * t + 8, 12 * h:12 * h + 16, :]
                blk = blk.reshape(128, N_TILES, N)
                tiles[:, t, h] = blk.transpose(1, 0, 2)
        # [nt, partition, (t,h)-block * N] so one contiguous DMA covers a
        # whole N-tile and per-block slices stay natural
        per_core.append(np.ascontiguousarray(
            tiles.transpose(0, 3, 1, 2, 4).reshape(N_TILES, 128, 12 * N)))
    return per_core


def kernel(**inputs):
    global LAST_EXEC_TIME_NS, LAST_RESULTS
    x = np.ascontiguousarray(np.asarray(inputs["x"], dtype=np.float32))
    wb16, wb32, scales = _host_prep(
        {k: np.asarray(v) for k, v in inputs.items()})

    nc = build_program(scales)

    xtiles = _host_tiles(x)
    in_maps = [{"xt": xtiles[i], "wb16": wb16, "wb32": wb32}
               for i in range(N_CORES)]

    trace = bool(os.environ.get("KERNEL_TRACE"))
    res = run_bass_kernel_spmd(nc, in_maps, list(range(N_CORES)), trace=trace)
    LAST_EXEC_TIME_NS = res.exec_time_ns
    LAST_RESULTS = res

    out = np.empty((B_TOTAL, 10), np.float32)
    for i in range(N_CORES):
        out[i * BC:(i + 1) * BC] = res.results[i]["out"].T
    return out
